# revision 26
# baseline (speedup 1.0000x reference)
"""Trainium2 Bass kernel: 2x2 zero-insertion upsample (dilate).

Full problem: x (16, 64, 256, 256) f32 -> out (16, 64, 512, 512) f32 with
out[..., 2i, 2j] = x[..., i, j], zeros elsewhere.

Strategy (memory-bound scatter, rel-err tolerance 2e-2):
- Shard batch dim across 8 cores: 2 batches/core.
- bf16 end-to-end on device (elementwise rel err <= 2^-9 ~ 0.2%, well under
  the 2e-2 gate): input converted f32->bf16 on host before DMA-in, output
  written bf16 and upcast on host.  Halves HBM traffic vs f32:
  16 MiB read + 32 MiB write per core -> ~141 us roofline at 358 GB/s
  per-core HBM.
- Column dilation via dtype trick: the bf16 input is viewed as uint16 and
  DVE tensor_copy's uint16->uint32 conversion zero-extends each element, so
  little-endian lane bytes [v0 v1 00 00] == bf16 pair (x, 0).  Unit-stride
  DVE, no odd-column memsets needed.
- Input row i maps to output row pair (2i dilated, 2i+1 zero).  Odd output
  rows and odd columns are never written: run_bass_kernel_spmd (native and
  bass2jax/PJRT paths) hands the kernel pre-zeroed ExternalOutput buffers
  (donated zero arrays), so skipping the zero writes halves HBM write
  traffic.
- Schedule: input tiles stream through a bounded ring of SWDGE (gpsimd)
  DMAs on their own queue (bounded so the big-packet input queue can't
  starve the 1KiB-descriptor output queues under the SDMA engines'
  packet-granularity round-robin); output DMAs alternate across the two
  HWDGE rings (SP/sync and Activation/scalar) because HWDGE descriptor
  generation (~2.5us + ~2ns/desc; out-DMAs have 2048 descriptors each)
  saturates a single ring before HBM saturates.
"""

import numpy as np
import ml_dtypes

BF16 = ml_dtypes.bfloat16

P = 128           # SBUF partitions
W = 256           # input row length (elements)
R = 16            # input rows per partition per tile
NBUF = 6          # out-slot pipeline depth
NBUF_IN = 6       # input ring depth (in t+6 paced by out t completion)
NROWS = 2 * 64 * 256          # input rows per core (batch-sharded: 2 of 16)
T = NROWS // (P * R)          # tiles per core
N_CORES = 8

_cache = {}


def _build_nc():
    import concourse.mybir as mybir
    import concourse.tile as tile
    from concourse import bacc

    u16 = mybir.dt.uint16
    u32 = mybir.dt.uint32
    nc = bacc.Bacc("TRN2", target_bir_lowering=False)
    x = nc.dram_tensor("x", (NROWS, W), u16, kind="ExternalInput")
    # u32 view of the bf16 output: y row i == output row pair (2i, 2i+1);
    # each u32 in [0:W) is a (data, zero) bf16 pair, [W:2W) stays zero.
    y = nc.dram_tensor("y", (NROWS, 2 * W), u32, kind="ExternalOutput")

    xv = x[:].rearrange("(t p r) w -> t p (r w)", p=P, r=R)
    yv = y[:].rearrange("(t p r) w -> t p r w", p=P, r=R)

    with tile.TileContext(nc) as tc:
        with tc.tile_pool(name="pin", bufs=NBUF_IN) as pin:
            for t in range(T):
                it = pin.tile([P, W * R], u16, tag="it", name=f"it{t}")
                # ins on the two HWDGE rings (cheap: 128 descriptors each)
                eng_in = nc.sync if t % 2 == 0 else nc.scalar
                eng_in.dma_start(it[:], xv[t])
                # u16 -> u32 zero-extend CAST DURING THE OUT-DMA (SWDGE
                # cast path): the DMA datapath itself inserts the bf16
                # zeros, so there is no DVE pass, no out staging tile, and
                # the SBUF-side out traffic is halved (16 MiB read instead
                # of 32) — the fabric stops being the binding resource
                nc.gpsimd.dma_start(
                    yv[t][:, :, 0:W],
                    it[:].rearrange("p (r w) -> p r w", w=W),
                )
    nc.finalize()
    return nc


def _run(x, trace=False):
    from concourse.bass_utils import run_bass_kernel_spmd

    if "nc" not in _cache:
        _cache["nc"] = _build_nc()
    nc = _cache["nc"]
    x = np.asarray(x, dtype=np.float32)
    per = x.shape[0] // N_CORES
    xb = x.astype(BF16).view(np.uint16)
    in_maps = [
        {"x": np.ascontiguousarray(xb[k * per : (k + 1) * per]).reshape(NROWS, W)}
        for k in range(N_CORES)
    ]
    try:
        res = run_bass_kernel_spmd(
            nc, in_maps, core_ids=list(range(N_CORES)), trace=trace
        )
    except Exception:
        # transient device wedge (e.g. NRT_EXEC_UNIT_UNRECOVERABLE) —
        # observed to clear on a clean re-execution; outputs are freshly
        # donated zero buffers per call, so a retry is a full re-run
        import os

        os.environ["NEURON_RT_RESET_CORES"] = "1"
        res = run_bass_kernel_spmd(
            nc, in_maps, core_ids=list(range(N_CORES)), trace=trace
        )
    parts = [
        res.results[k]["y"]
        .view(BF16)
        .reshape(per, 64, 512, 512)
        .astype(np.float32)
        for k in range(N_CORES)
    ]
    return np.concatenate(parts, axis=0), res


def kernel(**inputs) -> np.ndarray:
    out, _ = _run(inputs["x"])
    return out
